# revision 29
# baseline (speedup 1.0000x reference)
"""Trainium2 Bass kernel for nn_BaselineProt (embedding_lookup).

The reference computes, per drug-pair sample:
    multihot(drug) @ W0.T  ==  sum of W0 columns at the drug's (deduped)
    target proteins -- i.e. an embedding-table gather/sum, followed by a
    tiny MLP tower on each leg and a dot product between the two legs.

Structure (8 NeuronCores, data-parallel):
  Launch A: drugs sharded 500/core (padded to 512). The 16384 row-fetches
      per core are SWDGE descriptor-GEN bound (~994ns + ~7ns/idx per
      gather per queue, 4 queues in parallel -> ~2.2ns/idx sustained;
      HW-measured, insensitive to instruction sizing between 256 and 2048
      idxs per gather). 32 gathers of 512 idxs round-robin the 4 queues;
      per-gather partial trees + an interleaved accumulate chain reduce
      into the E shard while later gathers drain, leaving only the last
      gather's adds exposed. Q7 mlp-library IRAM load gates the first
      gather gen until ~18us; nothing else in launch A can absorb that.
  Host:     concatenates the 8 E shards into E [4096, 256] (pure data
      movement) and replicates it to every core.
  Launch B: batch sharded 1024 samples/core. Two packed const blob DMAs
      (w1/w2 on 128 partitions; cellb0 + leg-duplicated one-hot on 32)
      instead of seven strided loads. Transpose-mode E-row gather waves
      (8 x 256 idxs; 512-idx waves measured slower -- completions bunch)
      start as soon as the Q7 library lands. The leg-duplicated one-hot
      makes the PE cell matmul emit per-leg rows (contiguous PSUM->SBUF
      casts, no stride-2 dedup copies). The W1/W2/dot chains are
      software-pipelined per 512-leg tile in PE emission order so tile
      k's W2/dot never queue behind tile 3's W1 (PE queue is FIFO and
      tile 3 waits on the last wave); the +b2 step runs on DVE because
      the scalar engine paces the tail otherwise; the pair-dot is two
      512-sample bf16 ones-matmuls drained by the scalar engine.
"""

import os

os.environ.setdefault("JAX_PLATFORMS", "")

import numpy as np
import ml_dtypes

import concourse.bacc as bacc
import concourse.mybir as mybir
from concourse.tile import TileContext
from concourse import library_config
from concourse.bass_utils import run_bass_kernel_spmd

# Problem constants (hardcoded per harness contract).
B = 8192            # samples
P = 19000           # proteins
C = 32              # cell lines
D = 4000            # drugs
T = 32              # targets per drug
F = 256             # first hidden dim
H1 = 128            # second hidden dim
H2 = 64             # output dim per tower

NCORES = 8
DRUGS_PER_CORE = D // NCORES          # 500
DRUGS_PAD = 512                       # per-core padded drug count
SAMPLES_PER_CORE = B // NCORES        # 1024
ZROW = P + C                          # zero row in the W0T table (19032)
TAB_ROWS = ZROW + 8                   # pad table rows to 19040
E_ROWS = NCORES * DRUGS_PAD           # 4096 rows of E
NI_A = DRUGS_PAD * T                  # 16384 gather idxs per core, launch A
NI_B = 2 * SAMPLES_PER_CORE           # 2048 gather idxs per core, launch B
N_SUB = 4                             # launch A sub-batches (128 drugs each)
NG_A = 32                             # launch A gathers (512 idxs each)
N_ROUNDS = NG_A // N_SUB              # 8 rounds of 4 gathers (1 per queue)
NG_B = 8                              # launch B gather waves (256 idxs each)
NQ = 4                                # SWDGE queues

_BF16 = mybir.dt.bfloat16
_F32 = mybir.dt.float32
_I16 = mybir.dt.int16

# launch B const blobs (bf16 columns). Two blobs so the DMA only moves
# populated partitions: blob128 on all 128, blob32 on partitions 0-31.
_W1_OFF = 0                            # [128, 256]  w1t rearranged
_W2_OFF = 256                          # [128, 64]
_B128_W = 320
_CB_OFF = 0                            # [32, 256]   celltab (W0cell + b0)
_OH_OFF = 256                          # [32, 2048]  leg-duplicated one-hot
_B32_W = _OH_OFF + 2 * SAMPLES_PER_CORE    # 2304

_cache = {}


def _wrap_idx(flat):
    """Flat gather order -> the [128, n/16] int16 SBUF layout dma_gather
    expects (idx i at partition i%16, slot i//16; replicated to all 8 Q7
    core slices)."""
    n = flat.shape[0]
    assert n % 16 == 0
    arr = flat.astype(np.int16).reshape(n // 16, 16).T.copy()
    return np.tile(arr, (8, 1))


def _build_kernel_a():
    nc = bacc.Bacc("TRN2", target_bir_lowering=True, num_swdge_queues=NQ)
    tab = nc.dram_tensor("tab", [TAB_ROWS, F], _BF16, kind="ExternalInput")
    idxs = nc.dram_tensor("idxs", [128, NI_A // 16], _I16, kind="ExternalInput")
    e_out = nc.dram_tensor("e_out", [DRUGS_PAD, F], _BF16, kind="ExternalOutput")

    ni_s = NI_A // NG_A                           # idxs per gather (512)
    slots = T // N_ROUNDS                         # t-slots per gather (4)
    with TileContext(nc) as tc:
        nc.gpsimd.load_library(library_config.mlp)
        with (
            tc.tile_pool(name="idx", bufs=1) as ip,
            tc.tile_pool(name="g", bufs=1) as gp,
        ):
            idx_t = ip.tile([128, NI_A // 16], _I16)
            nc.sync.dma_start(out=idx_t[:, :], in_=idxs[:, :])
            # Gathers in rounds of 4 (one per queue, queue == sub-batch):
            # each sub-batch receives one 512-idx gather per round, so its
            # accumulate chain spreads across the whole gather phase and
            # the LAST round exposes only 4 gathers x 2 DVE ops (~3.5us)
            # instead of a whole sub-batch's reduction (~8us).
            acc2 = ip.tile([128, N_SUB, 2, F], _BF16)
            nc.vector.memset(acc2[:, :, :, :], 0.0)
            gs = [[None] * N_SUB for _ in range(N_ROUNDS)]
            for r in range(N_ROUNDS):
                for b in range(N_SUB):
                    g = gp.tile([128, slots, F], _BF16, tag=f"g{r}_{b}",
                                name=f"g{r}_{b}")
                    # flat idx range: [b*4096 + r*slots*128, +512)
                    c0 = (b * (T * 128) + r * slots * 128) // 16
                    nc.gpsimd.dma_gather(
                        g[:, :, :], tab[:],
                        idx_t[:, c0:c0 + ni_s // 16],
                        ni_s, ni_s, F,
                        single_packet=False, queue_num=b,
                    )
                    gs[r][b] = g
            # 2-op fold per gather, emitted in drain (round) order
            for r in range(N_ROUNDS):
                for b in range(N_SUB):
                    g = gs[r][b]
                    nc.vector.tensor_tensor(
                        out=g[:, 0:2, :], in0=g[:, 0:2, :],
                        in1=g[:, 2:4, :], op=mybir.AluOpType.add,
                    )
                    nc.vector.tensor_tensor(
                        out=acc2[:, b, :, :], in0=acc2[:, b, :, :],
                        in1=g[:, 0:2, :], op=mybir.AluOpType.add,
                    )
            # collapse the 2-wide accumulators (all sub-batches at once,
            # strided) and store E with a single rearranged DMA
            nc.vector.tensor_tensor(
                out=acc2[:, :, 0, :], in0=acc2[:, :, 0, :],
                in1=acc2[:, :, 1, :], op=mybir.AluOpType.add,
            )
            nc.sync.dma_start(
                out=e_out.ap().rearrange("(b p) f -> p b f", p=128),
                in_=acc2[:, :, 0, :],
            )
    nc.compile()
    return nc


def _build_kernel_b():
    nc = bacc.Bacc("TRN2", target_bir_lowering=True, num_swdge_queues=NQ)
    # NOTE: an SBUF-source (staged-E) variant of the gather was measured
    # 14us SLOWER than gathering straight from HBM -- transpose-mode
    # SBUF->SBUF DMA hits the xbar/SBUF-DMA serialization hazard
    etab = nc.dram_tensor("etab", [E_ROWS, F], _BF16, kind="ExternalInput")
    idxs = nc.dram_tensor("idxs", [128, NI_B // 16], _I16, kind="ExternalInput")
    blob = nc.dram_tensor("blob", [128, _B128_W], _BF16, kind="ExternalInput")
    blob32 = nc.dram_tensor("blob32", [C, _B32_W], _BF16, kind="ExternalInput")
    bias = nc.dram_tensor("bias", [128, 2], _F32, kind="ExternalInput")
    y = nc.dram_tensor("y", [1, SAMPLES_PER_CORE], _F32, kind="ExternalOutput")

    S = SAMPLES_PER_CORE                      # 1024
    L = 2 * S                                 # 2048 legs
    NT = 4                                    # matmul N tiles of 512
    TN = L // NT                              # 512
    SN = TN // 2                              # 256 samples per tile
    ni_g = NI_B // NG_B                       # 256 idxs per gather wave
    with TileContext(nc) as tc:
        nc.gpsimd.load_library(library_config.mlp)
        with (
            tc.tile_pool(name="const", bufs=1) as cp,
            tc.tile_pool(name="act", bufs=1) as ap,
            tc.tile_pool(name="ps1p", bufs=4, space="PSUM") as pp1,
            tc.tile_pool(name="ps", bufs=2, space="PSUM") as pp,
            tc.tile_pool(name="ps3p", bufs=1, space="PSUM") as pp3,
        ):
            # const loads first on the sync queue (they gate the cell
            # matmuls); idx last -- the gathers are library-load-gated
            # until ~18us anyway, and idx lands by ~12us.
            blob32_t = cp.tile([C, _B32_W], _BF16)
            nc.sync.dma_start(out=blob32_t[:, :], in_=blob32[:, :])
            blob_t = cp.tile([128, _B128_W], _BF16)
            nc.sync.dma_start(out=blob_t[:, :], in_=blob[:, :])
            bias_t = cp.tile([128, 2], _F32)
            nc.sync.dma_start(out=bias_t[:, :], in_=bias[:, :])
            idx_t = cp.tile([128, NI_B // 16], _I16)
            nc.sync.dma_start(out=idx_t[:, :], in_=idxs[:, :])

            # E-row gather: per sample s, rows E[d0], E[d1] at columns
            # 2s, 2s+1; feature-major via transpose mode.
            xts = []
            for g in range(NG_B):
                xt = ap.tile([128, 2, ni_g], _BF16, tag=f"xt{g}")
                nc.gpsimd.dma_gather(
                    xt[:, :, :], etab[:],
                    idx_t[:, g * (ni_g // 16):(g + 1) * (ni_g // 16)],
                    ni_g, ni_g, F,
                    # transpose-mode gathers corrupt with single_packet=False
                    transpose=True, single_packet=True, queue_num=g % NQ,
                )
                xts.append(xt)

            ones = cp.tile([64, 1], _BF16, tag="ones")
            nc.vector.memset(ones[:, :], 1.0)

            # per-leg cell+bias rows via one-hot matmul on the idle PE; the
            # one-hot is already leg-duplicated so the PSUM->SBUF casts are
            # contiguous.
            cells2 = ap.tile([128, 2, L], _BF16, tag="cells2")
            for fb in range(2):
                for q in range(NT):
                    psc = pp1.tile([128, TN], _F32, tag="ps1")
                    nc.tensor.matmul(
                        psc[:, :],
                        blob32_t[:, _CB_OFF + fb * 128:_CB_OFF + (fb + 1) * 128],
                        blob32_t[:, _OH_OFF + q * TN:_OH_OFF + (q + 1) * TN],
                        start=True, stop=True,
                    )
                    nc.vector.tensor_copy(
                        cells2[:, fb, q * TN:(q + 1) * TN], psc[:, :]
                    )

            # h0 = relu(E_leg + cellb0[s]) on DVE, bf16, contiguous ops.
            # add and relu are interleaved per wave: the DVE queue is FIFO,
            # so emitting all adds first would park wave 0's relu (which
            # gates the first matmul tile) behind wave 3's drain
            h0 = ap.tile([128, 2, L], _BF16, tag="h0")
            for g in range(NG_B):
                nc.vector.tensor_tensor(
                    out=h0[:, :, g * ni_g:(g + 1) * ni_g],
                    in0=xts[g][:, :, :],
                    in1=cells2[:, :, g * ni_g:(g + 1) * ni_g],
                    op=mybir.AluOpType.add,
                )
                nc.vector.tensor_scalar_max(
                    h0[:, :, g * ni_g:(g + 1) * ni_g],
                    h0[:, :, g * ni_g:(g + 1) * ni_g],
                    0.0,
                )

            h1 = ap.tile([128, L], _BF16, tag="h1")
            h2 = ap.tile([64, L], _F32, tag="h2")
            prod = ap.tile([64, S], _BF16, tag="prod")
            out_sb = ap.tile([1, S], _F32, tag="out")
            # Per-tile chains, software-pipelined in PE emission order so
            # tile k's W2/dot never queue behind tile 3's W1 (the PE queue
            # is FIFO; tile 3's W1 waits on the last gather wave). The
            # scalar relu/identity of tile k runs while PE works on tile
            # k+1's W1.
            ps1s = [pp1.tile([128, TN], _F32, tag="ps1", name=f"ps1_{i}")
                    for i in range(NT)]

            def emit_w1(nt):
                for c in range(2):
                    nc.tensor.matmul(
                        ps1s[nt][:, :],
                        blob_t[:, _W1_OFF + c * 128:_W1_OFF + (c + 1) * 128],
                        h0[:, c, nt * TN:(nt + 1) * TN],
                        start=(c == 0), stop=(c == 1),
                    )
                nc.scalar.activation(
                    h1[:, nt * TN:(nt + 1) * TN], ps1s[nt][:, :],
                    mybir.ActivationFunctionType.Relu,
                    bias=bias_t[:, 0:1], scale=1.0,
                )

            def emit_w2(nt):
                ps2 = pp.tile([64, TN], _F32, tag="ps2")
                nc.tensor.matmul(
                    ps2[:, :],
                    blob_t[:, _W2_OFF:_W2_OFF + H2],
                    h1[:, nt * TN:(nt + 1) * TN],
                    start=True, stop=True,
                )
                # + b2 on DVE (scalar engine is the tail pacer otherwise)
                nc.vector.tensor_scalar(
                    h2[:, nt * TN:(nt + 1) * TN], ps2[:, :],
                    bias_t[0:H2, 1:2], None, mybir.AluOpType.add,
                )
                # pair product for this tile's 256 samples
                nc.vector.tensor_tensor(
                    out=prod[:, nt * SN:(nt + 1) * SN],
                    in0=h2[:, nt * TN:(nt + 1) * TN:2],
                    in1=h2[:, nt * TN + 1:(nt + 1) * TN:2],
                    op=mybir.AluOpType.mult,
                )

            def emit_dot(half):
                # pair-dot over half the samples (512) in one bf16 matmul;
                # PSUM drained by the scalar engine (idle at the tail)
                ps3 = pp3.tile([1, 2 * SN], _F32, tag="ps3",
                               name=f"ps3_{half}", bufs=1)
                nc.tensor.matmul(
                    ps3[:, :], ones[:, :],
                    prod[:, half * 2 * SN:(half + 1) * 2 * SN],
                    start=True, stop=True,
                )
                nc.scalar.activation(
                    out_sb[:, half * 2 * SN:(half + 1) * 2 * SN], ps3[:, :],
                    mybir.ActivationFunctionType.Identity,
                    bias=0.0, scale=1.0,
                )

            emit_w1(0)
            emit_w1(1)
            emit_w2(0)
            emit_w1(2)
            emit_w2(1)
            emit_w1(3)
            emit_w2(2)
            emit_dot(0)
            emit_w2(3)
            emit_dot(1)
            nc.sync.dma_start(out=y[:, :], in_=out_sb[:, :])
    nc.compile()
    return nc


def _get_kernels():
    if "a" not in _cache:
        _cache["a"] = _build_kernel_a()
    if "b" not in _cache:
        _cache["b"] = _build_kernel_b()
    return _cache["a"], _cache["b"]


def _prep(drug_pairs, cell_lines, drug_targets, W0, b0, W1, b1, W2, b2):
    """Host-side data layout: shard, transpose, cast, build gather indices."""
    dt = np.asarray(drug_targets, dtype=np.int64)                  # [D, T]
    # dedup per row (reference uses .set -> dup targets count once)
    dup = (dt[:, :, None] == dt[:, None, :]) & (
        np.arange(T)[None, :, None] > np.arange(T)[None, None, :]
    )
    idx = np.where(dup.any(-1), ZROW, dt).astype(np.int32)          # [D, T]

    # W0T table: [P+C rows, F] bf16 + zero row + pad
    w0t = np.zeros((TAB_ROWS, F), dtype=ml_dtypes.bfloat16)
    w0t[: P + C] = np.asarray(W0, np.float32).T.astype(ml_dtypes.bfloat16)

    # launch A per-core gather index arrays
    idx_a = []
    for c in range(NCORES):
        rows = np.full((DRUGS_PAD, T), ZROW, np.int32)
        rows[:DRUGS_PER_CORE] = idx[c * DRUGS_PER_CORE:(c + 1) * DRUGS_PER_CORE]
        # flat j = b*4096 + t*128 + p  ->  drug 128b+p, target t
        flat = rows.reshape(4, 128, T).transpose(0, 2, 1).reshape(-1)
        idx_a.append(_wrap_idx(flat))

    # launch B per-core index arrays (E rows per leg) + const blobs
    dp = np.asarray(drug_pairs, dtype=np.int64)                     # [B, 2]
    cl = np.asarray(cell_lines, dtype=np.int64)                     # [B]
    e_row = (dp // DRUGS_PER_CORE) * DRUGS_PAD + (dp % DRUGS_PER_CORE)

    w1f = np.asarray(W1, np.float32)                                # [128, 256]
    w2f = np.asarray(W2, np.float32)                                # [64, 128]
    # cell rows with b0 folded in: relu input is E[d] + (W0cell[:,c] + b0)
    celltab = (
        np.asarray(W0, np.float32)[:, P:P + C].T + np.asarray(b0, np.float32)
    ).astype(ml_dtypes.bfloat16)                                    # [C, F]

    blob = np.zeros((128, _B128_W), dtype=ml_dtypes.bfloat16)
    # w1: blob[p, c*128+j] = W1T[c*128+p, j] = W1[j, c*128+p]
    for cblk in range(2):
        blob[:, _W1_OFF + cblk * 128:_W1_OFF + (cblk + 1) * 128] = (
            w1f[:, cblk * 128:(cblk + 1) * 128].T.astype(ml_dtypes.bfloat16)
        )
    blob[:, _W2_OFF:_W2_OFF + H2] = w2f.T.astype(ml_dtypes.bfloat16)

    blob32 = np.zeros((C, _B32_W), dtype=ml_dtypes.bfloat16)
    blob32[:, _CB_OFF:_CB_OFF + F] = celltab

    biasT = np.zeros((128, 2), dtype=np.float32)
    biasT[:, 0] = np.asarray(b1, np.float32)
    biasT[:H2, 1] = np.asarray(b2, np.float32)

    idx_b, blob32_b = [], []
    for c in range(NCORES):
        sl = slice(c * SAMPLES_PER_CORE, (c + 1) * SAMPLES_PER_CORE)
        idx_b.append(_wrap_idx(e_row[sl].reshape(-1)))
        bc = blob32.copy()
        # leg-duplicated one-hot: oh[cl[s], 2s+leg] = 1
        legs = np.repeat(cl[sl], 2)                                 # [2S]
        bc[legs, _OH_OFF + np.arange(2 * SAMPLES_PER_CORE)] = 1.0
        blob32_b.append(bc)

    return {
        "w0t": w0t, "idx_a": idx_a, "idx_b": idx_b,
        "blob": blob, "blob32_b": blob32_b, "biasT": biasT,
    }


def _run(inputs, trace=False):
    nca, ncb = _get_kernels()
    pr = _prep(**inputs)

    in_a = [{"tab": pr["w0t"], "idxs": pr["idx_a"][c]} for c in range(NCORES)]
    res_a = run_bass_kernel_spmd(
        nca, in_a, core_ids=list(range(NCORES)), trace=trace)

    e_ext = np.concatenate(
        [res_a.results[c]["e_out"] for c in range(NCORES)], axis=0
    )
    assert e_ext.shape == (E_ROWS, F)

    in_b = [
        {"etab": e_ext, "idxs": pr["idx_b"][c], "blob": pr["blob"],
         "blob32": pr["blob32_b"][c], "bias": pr["biasT"]}
        for c in range(NCORES)
    ]
    res_b = run_bass_kernel_spmd(
        ncb, in_b, core_ids=list(range(NCORES)), trace=trace)

    out = np.concatenate(
        [res_b.results[c]["y"].reshape(-1) for c in range(NCORES)]
    ).astype(np.float32)
    times = (res_a.exec_time_ns, res_b.exec_time_ns)
    return out, times


def kernel(**inputs) -> np.ndarray:
    out, _ = _run(inputs, trace=False)
    return out
